# revision 67
# baseline (speedup 1.0000x reference)
"""Trainium2 Bass kernel v3 for nn_AngularSymmetry.

Layout: j in partitions, (q_i, k) in free dim; chunk g covers i in
{g, g+32, g+64, g+96}.  Denominators fold into host-packed matmuls:

  MM1 (K=63): t0[j,(q,k)] = theta/(4pi*d_ij*d_ik) + c0, c0=0.25-NEUTRAL
  WMM (K=15, fp16 split): w[j,(q,k)] = ln(y_ij) + ln(y_ik), y = sqe/d
  ACT Sigmoid(scale=-1): GS = sigmoid(-w) = 1/(1 + y_ij*y_ik)  exactly
       the baseline's GS = 1/XP, via the log-separable form (scalar
       engine; sigmoid table set only, loaded once)
  DVE pass1 (ANGSYM_PA2): dlt = |t - round(t)| - 0.25,
       t = t0*GS + NEUTRAL   (x huge -> GS~0 -> t=NEUTRAL, whose p8
       equals the uniform-phase mean: unbiased chaotic-band handling)
  DVE pass2 (ANGSYM_PG): p8g = (((s+p)s+q)s+r) * GREPc3,  s = dlt^2
       monic cubic minimax fit of 2*cos(2*pi*dlt)^1.6 / c3; the c3
       leading coef is folded into GREPc3 = G_jk * c3 on the host.
       This replaces the Sin+Ln+Exp scalar passes AND the G_jk mul.
  PE reduce (per chunk, 4x): V[:,i_q] = sum_j p8g[j,(q,:)]*GT[:,i_q]
       (folds G_ij; V is a per-molecule [128,128] f32 PSUM)
  epilogue: W3 = V * GT (bf16) -> out_i = ones-matmul over k; DMA.
"""

import numpy as np

B, N = 16, 128
NCORES = 8
MPC = B // NCORES
EPS = 1e-5
SQE = float(np.sqrt(EPS))
FLOOR = 1e-9
NROW_TH = 21                   # 5*4 + const
NROW_X = 5                     # ones + 1 per q
K1 = 3 * NROW_TH               # 63
KX = 3 * NROW_X                # 15
CHUNK_J = 4
NCHUNK = N // CHUNK_J          # 32
SB = CHUNK_J * N               # 512
MAGIC = 12582912.0             # 1.5*2^23
NEUTRAL = 0.119140625          # bf16-exact; p8(0.25-NEUTRAL) = chaotic mean
C0VAL = 0.130859375            # 0.25 - NEUTRAL, bf16-exact

# monic cubic fit of H(s)=2*cos(2*pi*sqrt(s))^1.6 on s in [0,1/16]:
# H(s) ~ PC3*(s^3 + PCP*s^2 + PCQ*s + PCR); minimax err ~2.8e-3
PC3 = -507.46850634104544
PCP = -1.0095257669475188
PCQ = 0.12224718009976722
PCR = -0.003935608640825085

_PA_OP = None
_PG_OP = None
_GRAPH = None


def _register_op(name, spec, rd1_en):
    from concourse import dve_ops
    from concourse.dve_ops import DveOp
    from concourse.dve_spec import lower
    from concourse.dve_uop import DveOpSpec

    for op in dve_ops.OPS:
        if op.name == name:
            return op
    opcode = max(dve_ops._SUB_OPCODE_FOR_NAME.values()) + 1
    assert opcode < 0x20
    dve_ops._SUB_OPCODE_FOR_NAME[name] = opcode
    shas = {}
    for ver in ("v3", "v4"):
        try:
            uops = lower(spec, ver=ver)
            shas[ver] = DveOpSpec(
                name=name, opcode=opcode, uops=uops, rd1_en=rd1_en
            ).sha(ver)
        except Exception:
            pass
    assert shas, f"{name} failed to lower for all DVE versions"
    op = DveOp(name, spec, subdim=False, uops_sha=shas)
    dve_ops.OPS.append(op)
    dve_ops.CUSTOM_DVE_SPECS[name] = spec
    return op


def _make_pa_op():
    """dlt = |t - round(t)| - imm2, t = in0*in1 + s1, MAGIC in s0."""
    global _PA_OP
    if _PA_OP is not None:
        return _PA_OP
    from concourse.dve_spec import AluOp, Bin, C0, C1, Spec, Src0, Src1

    t = Src0 * Src1 + C1
    t2 = t + C0
    kk = t2 - C0
    dd = Bin(AluOp.ABSOLUTE_DIFF, t, kk)
    from concourse.dve_spec import C2
    body = dd - C2

    def _ref(in0, in1, s0, s1, imm2):
        f32 = np.float32
        t = (in0.astype(f32) * in1.astype(f32) + f32(s1)).astype(f32)
        t2 = (t + f32(s0)).astype(f32)
        kk = (t2 - f32(s0)).astype(f32)
        return (np.abs((t - kk).astype(f32)) - f32(imm2)).astype(f32)

    _PA_OP = _register_op("ANGSYM_PA2", Spec(body=body, reference=_ref), True)
    return _PA_OP


def _make_pg_op():
    """p8g = (((s+s0)*s+s1)*s+imm2) * in1, s = in0^2."""
    global _PG_OP
    if _PG_OP is not None:
        return _PG_OP
    from concourse.dve_spec import C0, C1, C2, Spec, Src0, Src1, sq

    s = sq(Src0)
    h = ((s + C0) * s + C1) * s + C2
    body = h * Src1

    def _ref(in0, in1, s0, s1, imm2):
        f32 = np.float32
        ss = (in0.astype(f32) * in0.astype(f32)).astype(f32)
        h = (ss + f32(s0)).astype(f32)
        h = (h * ss + f32(s1)).astype(f32)
        h = (h * ss + f32(imm2)).astype(f32)
        return (h * in1.astype(f32)).astype(f32)

    _PG_OP = _register_op("ANGSYM_PG", Spec(body=body, reference=_ref), True)
    return _PG_OP


def _host_precompute(d, dc, coords):
    """Pack per-molecule device feeds. d, dc: [N,N] f32; coords: [N,3]."""
    import ml_dtypes

    f32 = np.float32
    bf = ml_dtypes.bfloat16
    C = coords.astype(np.float64)
    S = (C @ C.T).astype(f32)
    diag = np.diag(S).copy()
    Cf = coords.astype(f32)
    G = (dc.astype(np.float64)
         * np.exp(-d.astype(np.float64) ** 2)).astype(f32)
    dcl = np.maximum(d.astype(f32), f32(FLOOR))
    rinv4 = (1.0 / (4.0 * np.pi * dcl)).astype(f32)
    rink = (1.0 / dcl).astype(f32)
    lny = np.log(f32(SQE) / dcl).astype(f32)   # in [-5.8, 15]; w = lny+lny

    L = np.zeros((NROW_TH, NCHUNK, N), f32)
    R = np.zeros((NROW_TH, NCHUNK, SB), f32)
    Lx = np.zeros((NROW_X, NCHUNK, N), f32)
    Rx = np.zeros((NROW_X, NCHUNK, SB), f32)
    Lx[0] = 1.0
    for g in range(NCHUNK):
        for q in range(CHUNK_J):
            i = g + NCHUNK * q
            ks = slice(q * N, (q + 1) * N)
            r0 = 5 * q
            L[r0 + 0, g, :] = (diag[i] - S[i, :]) * rinv4[i, :]
            R[r0 + 0, g, ks] = rink[i, :]
            for c in range(3):
                L[r0 + 1 + c, g, :] = Cf[:, c] * rinv4[i, :]
                R[r0 + 1 + c, g, ks] = Cf[:, c] * rink[i, :]
            L[r0 + 4, g, :] = rinv4[i, :]
            R[r0 + 4, g, ks] = -S[i, :] * rink[i, :]
            Lx[1 + q, g, :] = lny[i, :]
            Rx[1 + q, g, ks] = 1.0
            Rx[0, g, ks] = lny[i, :]
    L[20, :, :] = 1.0
    R[20, :, :] = C0VAL

    def split2(Lm, Rm, nr, dt):
        Lh = Lm.astype(dt)
        Ll = (Lm - Lh.astype(f32)).astype(dt)
        Rh = Rm.astype(dt)
        Rl = (Rm - Rh.astype(f32)).astype(dt)
        lhs = np.concatenate([Lh, Lh, Ll], axis=0)
        rhs = np.concatenate([Rh, Rl, Rh], axis=0)
        return (np.ascontiguousarray(lhs.reshape(3 * nr, NCHUNK * Lm.shape[2])),
                np.ascontiguousarray(rhs.reshape(3 * nr, NCHUNK * Rm.shape[2])))

    thl, thr = split2(L, R, NROW_TH, bf)
    xl, xr = split2(Lx, Rx, NROW_X, np.float16)
    g4 = np.tile((G * f32(PC3)).astype(bf), (1, CHUNK_J))
    return {
        "thl": thl, "thr": thr, "xl": xl, "xr": xr,
        "grep4": np.ascontiguousarray(g4),  # c3-scaled G_jk, pre-tiled 4x
        "gtb": np.ascontiguousarray(G.T).astype(bf),
    }


def emulate(d_cutoff, d, atom_coordinates):
    """Pure-numpy emulation of the device pipeline (for validation)."""
    import ml_dtypes

    bf = ml_dtypes.bfloat16
    f32 = np.float32
    f16 = np.float16
    out = np.zeros((B, N), f32)
    for b in range(B):
        pm = _host_precompute(
            np.asarray(d[b], f32), np.asarray(d_cutoff[b], f32),
            np.asarray(atom_coordinates[b], f32))
        thl = pm["thl"].astype(f32).reshape(K1, NCHUNK, N)
        thr = pm["thr"].astype(f32).reshape(K1, NCHUNK, SB)
        xl = pm["xl"].astype(f32).reshape(KX, NCHUNK, N)
        xr = pm["xr"].astype(f32).reshape(KX, NCHUNK, SB)
        GT = pm["gtb"].astype(f32)
        Gc3rep = pm["grep4"].astype(f32)
        V = np.zeros((N, N), f32)
        for g in range(NCHUNK):
            TH = np.einsum('mj,mc->jc', thl[:, g], thr[:, g]).astype(f32)
            WPm = np.einsum('mj,mc->jc', xl[:, g], xr[:, g]).astype(f32)
            GS = (f32(1.0) / (1.0 + np.exp(WPm))).astype(f32)  # sigmoid(-w)
            t = (TH * GS + f32(NEUTRAL)).astype(f32)
            kk = ((t + f32(MAGIC)).astype(f32) - f32(MAGIC)).astype(f32)
            dlt = (np.abs(t - kk) - f32(0.25)).astype(f16)
            ss = (dlt.astype(f32) ** 2).astype(f32)
            h = (ss + f32(PCP)).astype(f32)
            h = (h * ss + f32(PCQ)).astype(f32)
            h = (h * ss + f32(PCR)).astype(f32)
            p8g = (h * Gc3rep).astype(bf).astype(f32)
            for q in range(CHUNK_J):
                iq = g + NCHUNK * q
                V[:, iq] = p8g[:, q * N:(q + 1) * N].T @ GT[:, iq]
        W3 = (V * GT).astype(bf).astype(f32)
        out[b] = W3.sum(axis=0)
    return out


def build_graph(cfg=None):
    from contextlib import ExitStack

    import concourse.bass as bass
    import concourse.tile as tile
    from concourse import bacc, mybir
    from concourse.alu_op_type import AluOpType as ALU

    f32 = mybir.dt.float32
    bf16 = mybir.dt.bfloat16
    fp16 = mybir.dt.float16
    F = mybir.ActivationFunctionType

    pa_op = _make_pa_op()
    pg_op = _make_pg_op()

    nc = bacc.Bacc()
    thl_ext = nc.declare_dram_parameter("thl", [MPC, K1, NCHUNK * N], bf16, isOutput=False)
    thr_ext = nc.declare_dram_parameter("thr", [MPC, K1, NCHUNK * SB], bf16, isOutput=False)
    xl_ext = nc.declare_dram_parameter("xl", [MPC, KX, NCHUNK * N], fp16, isOutput=False)
    xr_ext = nc.declare_dram_parameter("xr", [MPC, KX, NCHUNK * SB], fp16, isOutput=False)
    grep_ext = nc.declare_dram_parameter("grep4", [MPC, N, SB], bf16, isOutput=False)
    gtb_ext = nc.declare_dram_parameter("gtb", [MPC, N, N], bf16, isOutput=False)
    out_ext = nc.declare_dram_parameter("out", [MPC, N], f32, isOutput=True)

    from concourse.hw_specs import get_activation_tables

    _tables = get_activation_tables(nc.m.arch)
    _sig_id = next(
        i for i, (nm, fs) in enumerate(_tables.items())
        if F.Sigmoid in fs
    )

    with ExitStack() as ctx:
        tc = ctx.enter_context(tile.TileContext(nc))
        consts = ctx.enter_context(tc.tile_pool(name="consts", bufs=1))
        molp = ctx.enter_context(tc.tile_pool(name="mol", bufs=2))
        bigp = ctx.enter_context(tc.tile_pool(name="big", bufs=2))
        psum_th = ctx.enter_context(tc.tile_pool(name="psum_th", bufs=4, space="PSUM"))
        psum_x = ctx.enter_context(tc.tile_pool(name="psum_x", bufs=3, space="PSUM"))
        psum_v = ctx.enter_context(tc.tile_pool(name="psum_v", bufs=1, space="PSUM"))
        gsp = ctx.enter_context(tc.tile_pool(name="gs", bufs=6))
        dltp = ctx.enter_context(tc.tile_pool(name="dlt", bufs=6))
        p8gp = ctx.enter_context(tc.tile_pool(name="p8g", bufs=6))

        _last_act = [None]

        def _chain(ins):
            from concourse.tile_rust import add_dep_helper
            if _last_act[0] is not None:
                add_dep_helper(ins, _last_act[0], sync=False, reason="act-order")
            _last_act[0] = ins

        def load_sig_table():
            inst = mybir.InstLoadActFuncSet(
                name=nc.get_next_instruction_name(), ins=[], outs=[],
                act_func_set_id=_sig_id,
            )
            bi = nc.scalar.add_instruction(inst)
            _chain(bi.ins)

        ones_bf = consts.tile([N, 1], bf16, tag="ones_bf")
        nc.vector.memset(ones_bf[:], 1.0)
        load_sig_table()

        mol_state = {}

        def emit_prologue(m):
            GTb = molp.tile([N, N], bf16, tag="GTb")
            nc.sync.dma_start(out=GTb[:], in_=gtb_ext[m])
            GREP = molp.tile([N, SB], bf16, tag="GREP")
            nc.sync.dma_start(out=GREP[:], in_=grep_ext[m])
            V = psum_v.tile([N, N], f32, tag="V")
            mol_state[m] = dict(GTb=GTb, GREP=GREP, V=V)

        def emit_part(m, g0, g1):
            if g0 == 0:
                emit_prologue(m)
            st = mol_state[m]
            nb = g1 - g0
            THL = bigp.tile([K1, nb * N], bf16, tag="THL")
            THR = bigp.tile([K1, nb * SB], bf16, tag="THR")
            XL = bigp.tile([KX, nb * N], fp16, tag="XL")
            XR = bigp.tile([KX, nb * SB], fp16, tag="XR")
            NQ = max(1, nb // 4)  # ~4-chunk DMA granules
            qn, qs = nb * N // NQ, nb * SB // NQ
            nc.sync.dma_start(out=XL[:], in_=xl_ext[m, :, g0 * N:g1 * N])
            for qd in range(NQ):
                nc.sync.dma_start(
                    out=XR[:, qd * qs:(qd + 1) * qs],
                    in_=xr_ext[m, :, g0 * SB + qd * qs:g0 * SB + (qd + 1) * qs])
                nc.sync.dma_start(
                    out=THL[:, qd * qn:(qd + 1) * qn],
                    in_=thl_ext[m, :, g0 * N + qd * qn:g0 * N + (qd + 1) * qn])
                nc.sync.dma_start(
                    out=THR[:, qd * qs:(qd + 1) * qs],
                    in_=thr_ext[m, :, g0 * SB + qd * qs:g0 * SB + (qd + 1) * qs])

            def emit_vr(g, P8G, base=0):
                for q in range(CHUNK_J):
                    iq = g + NCHUNK * q
                    nc.tensor.matmul(
                        out=st["V"][:, iq:iq + 1],
                        lhsT=P8G[:, base + q * N:base + (q + 1) * N],
                        rhs=st["GTb"][:, iq:iq + 1], start=True, stop=True)

            for g in range(g0, g1):
                lo_n, lo_s = (g - g0) * N, (g - g0) * SB
                WP = psum_x.tile([N, SB], f32, tag="WP")
                nc.tensor.matmul(
                    out=WP[:], lhsT=XL[:, lo_n:lo_n + N],
                    rhs=XR[:, lo_s:lo_s + SB], start=True, stop=True)
                TH = psum_th.tile([N, SB], f32, tag="TH")
                nc.tensor.matmul(
                    out=TH[:], lhsT=THL[:, lo_n:lo_n + N],
                    rhs=THR[:, lo_s:lo_s + SB], start=True, stop=True)
                GS = gsp.tile([N, SB], f32, tag="GS")
                bi = nc.scalar.activation(GS[:], WP[:], F.Sigmoid, scale=-1.0)
                _chain(bi.ins)
                dlt = dltp.tile([N, SB], fp16, tag="dlt")
                nc.vector._custom_dve(
                    pa_op, out=dlt[:], in0=TH[:], in1=GS[:],
                    s0=MAGIC, s1=NEUTRAL, imm2=0.25)
                P8G = p8gp.tile([N, SB], bf16, tag="P8G")
                nc.vector._custom_dve(
                    pg_op, out=P8G[:], in0=dlt[:], in1=st["GREP"][:],
                    s0=PCP, s1=PCQ, imm2=PCR)
                emit_vr(g, P8G)
            if g1 == NCHUNK:
                W3 = molp.tile([N, N], bf16, tag="W3")
                nc.vector.tensor_mul(out=W3[:], in0=st["V"][:], in1=st["GTb"][:])
                outr = molp.tile([N, N], f32, tag="outr")
                import bass_rust
                nc.gpsimd.partition_all_reduce(
                    outr[:], W3[:], N, bass_rust.ReduceOp.add)
                nc.sync.dma_start(out=out_ext[m], in_=outr[:1, :])

        HB = NCHUNK // 2
        for m in range(MPC):
            if m == 0:
                emit_part(m, 0, 1)
                emit_part(m, 1, 4)
                emit_part(m, 4, HB)
            else:
                emit_part(m, 0, HB)
            emit_part(m, HB, NCHUNK)

    return nc


def _get_graph():
    global _GRAPH
    if _GRAPH is None:
        _GRAPH = build_graph()
        _GRAPH.finalize()
    return _GRAPH


def make_in_maps(d_cutoff, d, atom_coordinates):
    in_maps = []
    for c in range(NCORES):
        per_mol = [
            _host_precompute(
                np.asarray(d[c * MPC + m], dtype=np.float32),
                np.asarray(d_cutoff[c * MPC + m], dtype=np.float32),
                np.asarray(atom_coordinates[c * MPC + m], dtype=np.float32),
            )
            for m in range(MPC)
        ]
        im = {
            k: np.ascontiguousarray(np.stack([pm[k] for pm in per_mol]))
            for k in per_mol[0]
        }
        in_maps.append(im)
    return in_maps


def kernel(d_cutoff, d, atom_coordinates):
    from concourse.bass_utils import run_bass_kernel_spmd

    nc = _get_graph()
    in_maps = make_in_maps(d_cutoff, d, atom_coordinates)
    res = run_bass_kernel_spmd(nc, in_maps, list(range(NCORES)))
    out = np.concatenate(
        [res.results[i]["out"] for i in range(NCORES)], axis=0
    ).astype(np.float32)
    return out


# revision 68
# speedup vs baseline: 1.0222x; 1.0222x over previous
"""Trainium2 Bass kernel v3 for nn_AngularSymmetry.

Layout: j in partitions, (q_i, k) in free dim; chunk g covers i in
{g, g+32, g+64, g+96}.  Denominators fold into host-packed matmuls:

  MM1 (K=63): t0[j,(q,k)] = theta/(4pi*d_ij*d_ik) + c0, c0=0.25-NEUTRAL
  WMM (K=15, fp16 split): w[j,(q,k)] = ln(y_ij) + ln(y_ik), y = sqe/d
  ACT Sigmoid(scale=-1): GS = sigmoid(-w) = 1/(1 + y_ij*y_ik)  exactly
       the baseline's GS = 1/XP, via the log-separable form (scalar
       engine; sigmoid table set only, loaded once)
  DVE pass1 (ANGSYM_PA2): dlt = |t - round(t)| - 0.25,
       t = t0*GS + NEUTRAL   (x huge -> GS~0 -> t=NEUTRAL, whose p8
       equals the uniform-phase mean: unbiased chaotic-band handling)
  DVE pass2 (ANGSYM_PG): p8g = (((s+p)s+q)s+r) * GREPc3,  s = dlt^2
       monic cubic minimax fit of 2*cos(2*pi*dlt)^1.6 / c3; the c3
       leading coef is folded into GREPc3 = G_jk * c3 on the host.
       This replaces the Sin+Ln+Exp scalar passes AND the G_jk mul.
  PE reduce (per chunk, 4x): V[:,i_q] = sum_j p8g[j,(q,:)]*GT[:,i_q]
       (folds G_ij; V is a per-molecule [128,128] f32 PSUM)
  epilogue: W3 = V * GT (bf16) -> out_i = ones-matmul over k; DMA.
"""

import numpy as np

B, N = 16, 128
NCORES = 8
MPC = B // NCORES
EPS = 1e-5
SQE = float(np.sqrt(EPS))
FLOOR = 1e-9
NROW_TH = 21                   # 5*4 + const
NROW_X = 5                     # ones + 1 per q
K1 = 3 * NROW_TH               # 63
KX = 3 * NROW_X                # 15
CHUNK_J = 4
NCHUNK = N // CHUNK_J          # 32
SB = CHUNK_J * N               # 512
MAGIC = 12582912.0             # 1.5*2^23
NEUTRAL = 0.119140625          # bf16-exact; p8(0.25-NEUTRAL) = chaotic mean
C0VAL = 0.130859375            # 0.25 - NEUTRAL, bf16-exact

# monic cubic fit of H(s)=2*cos(2*pi*sqrt(s))^1.6 on s in [0,1/16]:
# H(s) ~ PC3*(s^3 + PCP*s^2 + PCQ*s + PCR); minimax err ~2.8e-3
PC3 = -507.46850634104544
PCP = -1.0095257669475188
PCQ = 0.12224718009976722
PCR = -0.003935608640825085

_PA_OP = None
_PG_OP = None
_GRAPH = None


def _register_op(name, spec, rd1_en):
    from concourse import dve_ops
    from concourse.dve_ops import DveOp
    from concourse.dve_spec import lower
    from concourse.dve_uop import DveOpSpec

    for op in dve_ops.OPS:
        if op.name == name:
            return op
    opcode = max(dve_ops._SUB_OPCODE_FOR_NAME.values()) + 1
    assert opcode < 0x20
    dve_ops._SUB_OPCODE_FOR_NAME[name] = opcode
    shas = {}
    for ver in ("v3", "v4"):
        try:
            uops = lower(spec, ver=ver)
            shas[ver] = DveOpSpec(
                name=name, opcode=opcode, uops=uops, rd1_en=rd1_en
            ).sha(ver)
        except Exception:
            pass
    assert shas, f"{name} failed to lower for all DVE versions"
    op = DveOp(name, spec, subdim=False, uops_sha=shas)
    dve_ops.OPS.append(op)
    dve_ops.CUSTOM_DVE_SPECS[name] = spec
    return op


def _make_pa_op():
    """dlt = |t - round(t)| - imm2, t = in0*in1 + s1, MAGIC in s0."""
    global _PA_OP
    if _PA_OP is not None:
        return _PA_OP
    from concourse.dve_spec import AluOp, Bin, C0, C1, Spec, Src0, Src1

    t = Src0 * Src1 + C1
    t2 = t + C0
    kk = t2 - C0
    dd = Bin(AluOp.ABSOLUTE_DIFF, t, kk)
    from concourse.dve_spec import C2
    body = dd - C2

    def _ref(in0, in1, s0, s1, imm2):
        f32 = np.float32
        t = (in0.astype(f32) * in1.astype(f32) + f32(s1)).astype(f32)
        t2 = (t + f32(s0)).astype(f32)
        kk = (t2 - f32(s0)).astype(f32)
        return (np.abs((t - kk).astype(f32)) - f32(imm2)).astype(f32)

    _PA_OP = _register_op("ANGSYM_PA2", Spec(body=body, reference=_ref), True)
    return _PA_OP


def _make_pg_op():
    """p8g = (((s+s0)*s+s1)*s+imm2) * in1, s = in0^2."""
    global _PG_OP
    if _PG_OP is not None:
        return _PG_OP
    from concourse.dve_spec import C0, C1, C2, Spec, Src0, Src1, sq

    s = sq(Src0)
    h = ((s + C0) * s + C1) * s + C2
    body = h * Src1

    def _ref(in0, in1, s0, s1, imm2):
        f32 = np.float32
        ss = (in0.astype(f32) * in0.astype(f32)).astype(f32)
        h = (ss + f32(s0)).astype(f32)
        h = (h * ss + f32(s1)).astype(f32)
        h = (h * ss + f32(imm2)).astype(f32)
        return (h * in1.astype(f32)).astype(f32)

    _PG_OP = _register_op("ANGSYM_PG", Spec(body=body, reference=_ref), True)
    return _PG_OP


def _host_precompute(d, dc, coords):
    """Pack per-molecule device feeds. d, dc: [N,N] f32; coords: [N,3]."""
    import ml_dtypes

    f32 = np.float32
    bf = ml_dtypes.bfloat16
    C = coords.astype(np.float64)
    S = (C @ C.T).astype(f32)
    diag = np.diag(S).copy()
    Cf = coords.astype(f32)
    G = (dc.astype(np.float64)
         * np.exp(-d.astype(np.float64) ** 2)).astype(f32)
    dcl = np.maximum(d.astype(f32), f32(FLOOR))
    rinv4 = (1.0 / (4.0 * np.pi * dcl)).astype(f32)
    rink = (1.0 / dcl).astype(f32)
    lny = np.log(f32(SQE) / dcl).astype(f32)   # in [-5.8, 15]; w = lny+lny

    L = np.zeros((NROW_TH, NCHUNK, N), f32)
    R = np.zeros((NROW_TH, NCHUNK, SB), f32)
    Lx = np.zeros((NROW_X, NCHUNK, N), f32)
    Rx = np.zeros((NROW_X, NCHUNK, SB), f32)
    Lx[0] = 1.0
    for g in range(NCHUNK):
        for q in range(CHUNK_J):
            i = g + NCHUNK * q
            ks = slice(q * N, (q + 1) * N)
            r0 = 5 * q
            L[r0 + 0, g, :] = (diag[i] - S[i, :]) * rinv4[i, :]
            R[r0 + 0, g, ks] = rink[i, :]
            for c in range(3):
                L[r0 + 1 + c, g, :] = Cf[:, c] * rinv4[i, :]
                R[r0 + 1 + c, g, ks] = Cf[:, c] * rink[i, :]
            L[r0 + 4, g, :] = rinv4[i, :]
            R[r0 + 4, g, ks] = -S[i, :] * rink[i, :]
            Lx[1 + q, g, :] = lny[i, :]
            Rx[1 + q, g, ks] = 1.0
            Rx[0, g, ks] = lny[i, :]
    L[20, :, :] = 1.0
    R[20, :, :] = C0VAL

    def split2(Lm, Rm, nr, dt):
        Lh = Lm.astype(dt)
        Ll = (Lm - Lh.astype(f32)).astype(dt)
        Rh = Rm.astype(dt)
        Rl = (Rm - Rh.astype(f32)).astype(dt)
        lhs = np.concatenate([Lh, Lh, Ll], axis=0)
        rhs = np.concatenate([Rh, Rl, Rh], axis=0)
        return (np.ascontiguousarray(lhs.reshape(3 * nr, NCHUNK * Lm.shape[2])),
                np.ascontiguousarray(rhs.reshape(3 * nr, NCHUNK * Rm.shape[2])))

    thl, thr = split2(L, R, NROW_TH, bf)
    xl, xr = split2(Lx, Rx, NROW_X, np.float16)
    g4 = np.tile((G * f32(PC3)).astype(bf), (1, CHUNK_J))
    return {
        "thl": thl, "thr": thr, "xl": xl, "xr": xr,
        "grep4": np.ascontiguousarray(g4),  # c3-scaled G_jk, pre-tiled 4x
        "gtb": np.ascontiguousarray(G.T).astype(bf),
    }


def emulate(d_cutoff, d, atom_coordinates):
    """Pure-numpy emulation of the device pipeline (for validation)."""
    import ml_dtypes

    bf = ml_dtypes.bfloat16
    f32 = np.float32
    f16 = np.float16
    out = np.zeros((B, N), f32)
    for b in range(B):
        pm = _host_precompute(
            np.asarray(d[b], f32), np.asarray(d_cutoff[b], f32),
            np.asarray(atom_coordinates[b], f32))
        thl = pm["thl"].astype(f32).reshape(K1, NCHUNK, N)
        thr = pm["thr"].astype(f32).reshape(K1, NCHUNK, SB)
        xl = pm["xl"].astype(f32).reshape(KX, NCHUNK, N)
        xr = pm["xr"].astype(f32).reshape(KX, NCHUNK, SB)
        GT = pm["gtb"].astype(f32)
        Gc3rep = pm["grep4"].astype(f32)
        V = np.zeros((N, N), f32)
        for g in range(NCHUNK):
            TH = np.einsum('mj,mc->jc', thl[:, g], thr[:, g]).astype(f32)
            WPm = np.einsum('mj,mc->jc', xl[:, g], xr[:, g]).astype(f32)
            GS = (f32(1.0) / (1.0 + np.exp(WPm))).astype(f32)  # sigmoid(-w)
            t = (TH * GS + f32(NEUTRAL)).astype(f32)
            kk = ((t + f32(MAGIC)).astype(f32) - f32(MAGIC)).astype(f32)
            dlt = (np.abs(t - kk) - f32(0.25)).astype(f16)
            ss = (dlt.astype(f32) ** 2).astype(f32)
            h = (ss + f32(PCP)).astype(f32)
            h = (h * ss + f32(PCQ)).astype(f32)
            h = (h * ss + f32(PCR)).astype(f32)
            p8g = (h * Gc3rep).astype(bf).astype(f32)
            for q in range(CHUNK_J):
                iq = g + NCHUNK * q
                V[:, iq] = p8g[:, q * N:(q + 1) * N].T @ GT[:, iq]
        W3 = (V * GT).astype(bf).astype(f32)
        out[b] = W3.sum(axis=0)
    return out


def build_graph(cfg=None):
    from contextlib import ExitStack

    import concourse.bass as bass
    import concourse.tile as tile
    from concourse import bacc, mybir
    from concourse.alu_op_type import AluOpType as ALU

    f32 = mybir.dt.float32
    bf16 = mybir.dt.bfloat16
    fp16 = mybir.dt.float16
    F = mybir.ActivationFunctionType

    pa_op = _make_pa_op()
    pg_op = _make_pg_op()

    nc = bacc.Bacc()
    thl_ext = nc.declare_dram_parameter("thl", [MPC, K1, NCHUNK * N], bf16, isOutput=False)
    thr_ext = nc.declare_dram_parameter("thr", [MPC, K1, NCHUNK * SB], bf16, isOutput=False)
    xl_ext = nc.declare_dram_parameter("xl", [MPC, KX, NCHUNK * N], fp16, isOutput=False)
    xr_ext = nc.declare_dram_parameter("xr", [MPC, KX, NCHUNK * SB], fp16, isOutput=False)
    grep_ext = nc.declare_dram_parameter("grep4", [MPC, N, SB], bf16, isOutput=False)
    gtb_ext = nc.declare_dram_parameter("gtb", [MPC, N, N], bf16, isOutput=False)
    out_ext = nc.declare_dram_parameter("out", [MPC, N], f32, isOutput=True)

    from concourse.hw_specs import get_activation_tables

    _tables = get_activation_tables(nc.m.arch)
    _sig_id = next(
        i for i, (nm, fs) in enumerate(_tables.items())
        if F.Sigmoid in fs
    )

    with ExitStack() as ctx:
        tc = ctx.enter_context(tile.TileContext(nc))
        consts = ctx.enter_context(tc.tile_pool(name="consts", bufs=1))
        molp = ctx.enter_context(tc.tile_pool(name="mol", bufs=2))
        bigp = ctx.enter_context(tc.tile_pool(name="big", bufs=2))
        psum_th = ctx.enter_context(tc.tile_pool(name="psum_th", bufs=4, space="PSUM"))
        psum_x = ctx.enter_context(tc.tile_pool(name="psum_x", bufs=3, space="PSUM"))
        psum_v = ctx.enter_context(tc.tile_pool(name="psum_v", bufs=1, space="PSUM"))
        gsp = ctx.enter_context(tc.tile_pool(name="gs", bufs=6))
        dltp = ctx.enter_context(tc.tile_pool(name="dlt", bufs=6))
        p8gp = ctx.enter_context(tc.tile_pool(name="p8g", bufs=6))

        _last_act = [None]

        def _chain(ins):
            from concourse.tile_rust import add_dep_helper
            if _last_act[0] is not None:
                add_dep_helper(ins, _last_act[0], sync=False, reason="act-order")
            _last_act[0] = ins

        def load_sig_table():
            inst = mybir.InstLoadActFuncSet(
                name=nc.get_next_instruction_name(), ins=[], outs=[],
                act_func_set_id=_sig_id,
            )
            bi = nc.scalar.add_instruction(inst)
            _chain(bi.ins)

        ones_bf = consts.tile([N, 1], bf16, tag="ones_bf")
        nc.vector.memset(ones_bf[:], 1.0)
        load_sig_table()

        mol_state = {}

        def emit_prologue(m):
            GTb = molp.tile([N, N], bf16, tag="GTb")
            nc.sync.dma_start(out=GTb[:], in_=gtb_ext[m])
            GREP = molp.tile([N, SB], bf16, tag="GREP")
            nc.sync.dma_start(out=GREP[:], in_=grep_ext[m])
            V = psum_v.tile([N, N], f32, tag="V")
            mol_state[m] = dict(GTb=GTb, GREP=GREP, V=V)

        def emit_part(m, g0, g1):
            if g0 == 0:
                emit_prologue(m)
            st = mol_state[m]
            nb = g1 - g0
            THL = bigp.tile([K1, nb * N], bf16, tag="THL")
            THR = bigp.tile([K1, nb * SB], bf16, tag="THR")
            XL = bigp.tile([KX, nb * N], fp16, tag="XL")
            XR = bigp.tile([KX, nb * SB], fp16, tag="XR")
            NQ = max(1, nb // 4)  # ~4-chunk DMA granules
            qn, qs = nb * N // NQ, nb * SB // NQ
            nc.sync.dma_start(out=XL[:], in_=xl_ext[m, :, g0 * N:g1 * N])
            for qd in range(NQ):
                nc.sync.dma_start(
                    out=XR[:, qd * qs:(qd + 1) * qs],
                    in_=xr_ext[m, :, g0 * SB + qd * qs:g0 * SB + (qd + 1) * qs])
                nc.sync.dma_start(
                    out=THL[:, qd * qn:(qd + 1) * qn],
                    in_=thl_ext[m, :, g0 * N + qd * qn:g0 * N + (qd + 1) * qn])
                nc.sync.dma_start(
                    out=THR[:, qd * qs:(qd + 1) * qs],
                    in_=thr_ext[m, :, g0 * SB + qd * qs:g0 * SB + (qd + 1) * qs])

            def emit_vr(g, P8G, base=0):
                for q in range(CHUNK_J):
                    iq = g + NCHUNK * q
                    nc.tensor.matmul(
                        out=st["V"][:, iq:iq + 1],
                        lhsT=P8G[:, base + q * N:base + (q + 1) * N],
                        rhs=st["GTb"][:, iq:iq + 1], start=True, stop=True)

            for g in range(g0, g1):
                lo_n, lo_s = (g - g0) * N, (g - g0) * SB
                WP = psum_x.tile([N, SB], f32, tag="WP")
                nc.tensor.matmul(
                    out=WP[:], lhsT=XL[:, lo_n:lo_n + N],
                    rhs=XR[:, lo_s:lo_s + SB], start=True, stop=True)
                TH = psum_th.tile([N, SB], f32, tag="TH")
                nc.tensor.matmul(
                    out=TH[:], lhsT=THL[:, lo_n:lo_n + N],
                    rhs=THR[:, lo_s:lo_s + SB], start=True, stop=True)
                GS = gsp.tile([N, SB], f32, tag="GS")
                bi = nc.scalar.activation(GS[:], WP[:], F.Sigmoid, scale=-1.0)
                _chain(bi.ins)
                dlt = dltp.tile([N, SB], fp16, tag="dlt")
                nc.vector._custom_dve(
                    pa_op, out=dlt[:], in0=TH[:], in1=GS[:],
                    s0=MAGIC, s1=NEUTRAL, imm2=0.25)
                P8G = p8gp.tile([N, SB], bf16, tag="P8G")
                nc.vector._custom_dve(
                    pg_op, out=P8G[:], in0=dlt[:], in1=st["GREP"][:],
                    s0=PCP, s1=PCQ, imm2=PCR)
                emit_vr(g, P8G)
            if g1 == NCHUNK:
                W3 = molp.tile([N, N], bf16, tag="W3")
                nc.vector.tensor_mul(out=W3[:], in0=st["V"][:], in1=st["GTb"][:])
                outr = molp.tile([N, N], f32, tag="outr")
                import bass_rust
                nc.gpsimd.partition_all_reduce(
                    outr[:], W3[:], N, bass_rust.ReduceOp.add)
                nc.sync.dma_start(out=out_ext[m], in_=outr[:1, :])

        HB = NCHUNK // 2
        for m in range(MPC):
            if m == 0:
                emit_part(m, 0, 4)
                emit_part(m, 4, HB)
            else:
                emit_part(m, 0, HB)
            emit_part(m, HB, NCHUNK)

    return nc


def _get_graph():
    global _GRAPH
    if _GRAPH is None:
        _GRAPH = build_graph()
        _GRAPH.finalize()
    return _GRAPH


def make_in_maps(d_cutoff, d, atom_coordinates):
    in_maps = []
    for c in range(NCORES):
        per_mol = [
            _host_precompute(
                np.asarray(d[c * MPC + m], dtype=np.float32),
                np.asarray(d_cutoff[c * MPC + m], dtype=np.float32),
                np.asarray(atom_coordinates[c * MPC + m], dtype=np.float32),
            )
            for m in range(MPC)
        ]
        im = {
            k: np.ascontiguousarray(np.stack([pm[k] for pm in per_mol]))
            for k in per_mol[0]
        }
        in_maps.append(im)
    return in_maps


def kernel(d_cutoff, d, atom_coordinates):
    from concourse.bass_utils import run_bass_kernel_spmd

    nc = _get_graph()
    in_maps = make_in_maps(d_cutoff, d, atom_coordinates)
    res = run_bass_kernel_spmd(nc, in_maps, list(range(NCORES)))
    out = np.concatenate(
        [res.results[i]["out"] for i in range(NCORES)], axis=0
    ).astype(np.float32)
    return out


# revision 69
# speedup vs baseline: 1.0389x; 1.0163x over previous
"""Trainium2 Bass kernel v3 for nn_AngularSymmetry.

Layout: j in partitions, (q_i, k) in free dim; chunk g covers i in
{g, g+32, g+64, g+96}.  Denominators fold into host-packed matmuls:

  MM1 (K=63): t0[j,(q,k)] = theta/(4pi*d_ij*d_ik) + c0, c0=0.25-NEUTRAL
  WMM (K=15, fp16 split): w[j,(q,k)] = ln(y_ij) + ln(y_ik), y = sqe/d
  ACT Sigmoid(scale=-1): GS = sigmoid(-w) = 1/(1 + y_ij*y_ik)  exactly
       the baseline's GS = 1/XP, via the log-separable form (scalar
       engine; sigmoid table set only, loaded once)
  DVE pass1 (ANGSYM_PA2): dlt = |t - round(t)| - 0.25,
       t = t0*GS + NEUTRAL   (x huge -> GS~0 -> t=NEUTRAL, whose p8
       equals the uniform-phase mean: unbiased chaotic-band handling)
  DVE pass2 (ANGSYM_PG): p8g = (((s+p)s+q)s+r) * GREPc3,  s = dlt^2
       monic cubic minimax fit of 2*cos(2*pi*dlt)^1.6 / c3; the c3
       leading coef is folded into GREPc3 = G_jk * c3 on the host.
       This replaces the Sin+Ln+Exp scalar passes AND the G_jk mul.
  PE reduce (per chunk, 4x): V[:,i_q] = sum_j p8g[j,(q,:)]*GT[:,i_q]
       (folds G_ij; V is a per-molecule [128,128] f32 PSUM)
  epilogue: W3 = V * GT (bf16) -> out_i = ones-matmul over k; DMA.
"""

import numpy as np

B, N = 16, 128
NCORES = 8
MPC = B // NCORES
EPS = 1e-5
SQE = float(np.sqrt(EPS))
FLOOR = 1e-9
NROW_TH = 21                   # 5*4 + const
NROW_X = 5                     # ones + 1 per q
K1 = 3 * NROW_TH               # 63
KX = 3 * NROW_X                # 15
CHUNK_J = 4
NCHUNK = N // CHUNK_J          # 32
SB = CHUNK_J * N               # 512
MAGIC = 12582912.0             # 1.5*2^23
NEUTRAL = 0.119140625          # bf16-exact; p8(0.25-NEUTRAL) = chaotic mean
C0VAL = 0.130859375            # 0.25 - NEUTRAL, bf16-exact

# monic cubic fit of H(s)=2*cos(2*pi*sqrt(s))^1.6 on s in [0,1/16]:
# H(s) ~ PC3*(s^3 + PCP*s^2 + PCQ*s + PCR); minimax err ~2.8e-3
PC3 = -507.46850634104544
PCP = -1.0095257669475188
PCQ = 0.12224718009976722
PCR = -0.003935608640825085

_PA_OP = None
_PG_OP = None
_GRAPH = None


def _register_op(name, spec, rd1_en):
    from concourse import dve_ops
    from concourse.dve_ops import DveOp
    from concourse.dve_spec import lower
    from concourse.dve_uop import DveOpSpec

    for op in dve_ops.OPS:
        if op.name == name:
            return op
    opcode = max(dve_ops._SUB_OPCODE_FOR_NAME.values()) + 1
    assert opcode < 0x20
    dve_ops._SUB_OPCODE_FOR_NAME[name] = opcode
    shas = {}
    for ver in ("v3", "v4"):
        try:
            uops = lower(spec, ver=ver)
            shas[ver] = DveOpSpec(
                name=name, opcode=opcode, uops=uops, rd1_en=rd1_en
            ).sha(ver)
        except Exception:
            pass
    assert shas, f"{name} failed to lower for all DVE versions"
    op = DveOp(name, spec, subdim=False, uops_sha=shas)
    dve_ops.OPS.append(op)
    dve_ops.CUSTOM_DVE_SPECS[name] = spec
    return op


def _make_pa_op():
    """dlt = |t - round(t)| - imm2, t = in0*in1 + s1, MAGIC in s0."""
    global _PA_OP
    if _PA_OP is not None:
        return _PA_OP
    from concourse.dve_spec import AluOp, Bin, C0, C1, Spec, Src0, Src1

    t = Src0 * Src1 + C1
    t2 = t + C0
    kk = t2 - C0
    dd = Bin(AluOp.ABSOLUTE_DIFF, t, kk)
    from concourse.dve_spec import C2
    body = dd - C2

    def _ref(in0, in1, s0, s1, imm2):
        f32 = np.float32
        t = (in0.astype(f32) * in1.astype(f32) + f32(s1)).astype(f32)
        t2 = (t + f32(s0)).astype(f32)
        kk = (t2 - f32(s0)).astype(f32)
        return (np.abs((t - kk).astype(f32)) - f32(imm2)).astype(f32)

    _PA_OP = _register_op("ANGSYM_PA2", Spec(body=body, reference=_ref), True)
    return _PA_OP


def _make_pg_op():
    """p8g = (((s+s0)*s+s1)*s+imm2) * in1, s = in0^2."""
    global _PG_OP
    if _PG_OP is not None:
        return _PG_OP
    from concourse.dve_spec import C0, C1, C2, Spec, Src0, Src1, sq

    s = sq(Src0)
    h = ((s + C0) * s + C1) * s + C2
    body = h * Src1

    def _ref(in0, in1, s0, s1, imm2):
        f32 = np.float32
        ss = (in0.astype(f32) * in0.astype(f32)).astype(f32)
        h = (ss + f32(s0)).astype(f32)
        h = (h * ss + f32(s1)).astype(f32)
        h = (h * ss + f32(imm2)).astype(f32)
        return (h * in1.astype(f32)).astype(f32)

    _PG_OP = _register_op("ANGSYM_PG", Spec(body=body, reference=_ref), True)
    return _PG_OP


def _host_precompute(d, dc, coords):
    """Pack per-molecule device feeds. d, dc: [N,N] f32; coords: [N,3]."""
    import ml_dtypes

    f32 = np.float32
    bf = ml_dtypes.bfloat16
    C = coords.astype(np.float64)
    S = (C @ C.T).astype(f32)
    diag = np.diag(S).copy()
    Cf = coords.astype(f32)
    G = (dc.astype(np.float64)
         * np.exp(-d.astype(np.float64) ** 2)).astype(f32)
    dcl = np.maximum(d.astype(f32), f32(FLOOR))
    rinv4 = (1.0 / (4.0 * np.pi * dcl)).astype(f32)
    rink = (1.0 / dcl).astype(f32)
    lny = np.log(f32(SQE) / dcl).astype(f32)   # in [-5.8, 15]; w = lny+lny

    L = np.zeros((NROW_TH, NCHUNK, N), f32)
    R = np.zeros((NROW_TH, NCHUNK, SB), f32)
    Lx = np.zeros((NROW_X, NCHUNK, N), f32)
    Rx = np.zeros((NROW_X, NCHUNK, SB), f32)
    Lx[0] = 1.0
    for g in range(NCHUNK):
        for q in range(CHUNK_J):
            i = g + NCHUNK * q
            ks = slice(q * N, (q + 1) * N)
            r0 = 5 * q
            L[r0 + 0, g, :] = (diag[i] - S[i, :]) * rinv4[i, :]
            R[r0 + 0, g, ks] = rink[i, :]
            for c in range(3):
                L[r0 + 1 + c, g, :] = Cf[:, c] * rinv4[i, :]
                R[r0 + 1 + c, g, ks] = Cf[:, c] * rink[i, :]
            L[r0 + 4, g, :] = rinv4[i, :]
            R[r0 + 4, g, ks] = -S[i, :] * rink[i, :]
            Lx[1 + q, g, :] = lny[i, :]
            Rx[1 + q, g, ks] = 1.0
            Rx[0, g, ks] = lny[i, :]
    L[20, :, :] = 1.0
    R[20, :, :] = C0VAL

    def split2(Lm, Rm, nr, dt):
        Lh = Lm.astype(dt)
        Ll = (Lm - Lh.astype(f32)).astype(dt)
        Rh = Rm.astype(dt)
        Rl = (Rm - Rh.astype(f32)).astype(dt)
        lhs = np.concatenate([Lh, Lh, Ll], axis=0)
        rhs = np.concatenate([Rh, Rl, Rh], axis=0)
        return (np.ascontiguousarray(lhs.reshape(3 * nr, NCHUNK * Lm.shape[2])),
                np.ascontiguousarray(rhs.reshape(3 * nr, NCHUNK * Rm.shape[2])))

    thl, thr = split2(L, R, NROW_TH, bf)
    xl, xr = split2(Lx, Rx, NROW_X, np.float16)
    g4 = np.tile((G * f32(PC3)).astype(bf), (1, 2 * CHUNK_J))
    return {
        "thl": thl, "thr": thr, "xl": xl, "xr": xr,
        "grep4": np.ascontiguousarray(g4),  # c3-scaled G_jk, pre-tiled 8x
        "gtb": np.ascontiguousarray(G.T).astype(bf),
    }


def emulate(d_cutoff, d, atom_coordinates):
    """Pure-numpy emulation of the device pipeline (for validation)."""
    import ml_dtypes

    bf = ml_dtypes.bfloat16
    f32 = np.float32
    f16 = np.float16
    out = np.zeros((B, N), f32)
    for b in range(B):
        pm = _host_precompute(
            np.asarray(d[b], f32), np.asarray(d_cutoff[b], f32),
            np.asarray(atom_coordinates[b], f32))
        thl = pm["thl"].astype(f32).reshape(K1, NCHUNK, N)
        thr = pm["thr"].astype(f32).reshape(K1, NCHUNK, SB)
        xl = pm["xl"].astype(f32).reshape(KX, NCHUNK, N)
        xr = pm["xr"].astype(f32).reshape(KX, NCHUNK, SB)
        GT = pm["gtb"].astype(f32)
        Gc3rep = pm["grep4"].astype(f32)[:, :SB]
        V = np.zeros((N, N), f32)
        for g in range(NCHUNK):
            TH = np.einsum('mj,mc->jc', thl[:, g], thr[:, g]).astype(f32)
            WPm = np.einsum('mj,mc->jc', xl[:, g], xr[:, g]).astype(f32)
            GS = (f32(1.0) / (1.0 + np.exp(WPm))).astype(f32)  # sigmoid(-w)
            t = (TH * GS + f32(NEUTRAL)).astype(f32)
            kk = ((t + f32(MAGIC)).astype(f32) - f32(MAGIC)).astype(f32)
            dlt = (np.abs(t - kk) - f32(0.25)).astype(f16)
            ss = (dlt.astype(f32) ** 2).astype(f32)
            h = (ss + f32(PCP)).astype(f32)
            h = (h * ss + f32(PCQ)).astype(f32)
            h = (h * ss + f32(PCR)).astype(f32)
            p8g = (h * Gc3rep).astype(bf).astype(f32)
            for q in range(CHUNK_J):
                iq = g + NCHUNK * q
                V[:, iq] = p8g[:, q * N:(q + 1) * N].T @ GT[:, iq]
        W3 = (V * GT).astype(bf).astype(f32)
        out[b] = W3.sum(axis=0)
    return out


def build_graph(cfg=None):
    from contextlib import ExitStack

    import concourse.bass as bass
    import concourse.tile as tile
    from concourse import bacc, mybir
    from concourse.alu_op_type import AluOpType as ALU

    f32 = mybir.dt.float32
    bf16 = mybir.dt.bfloat16
    fp16 = mybir.dt.float16
    F = mybir.ActivationFunctionType

    pa_op = _make_pa_op()
    pg_op = _make_pg_op()

    nc = bacc.Bacc()
    thl_ext = nc.declare_dram_parameter("thl", [MPC, K1, NCHUNK * N], bf16, isOutput=False)
    thr_ext = nc.declare_dram_parameter("thr", [MPC, K1, NCHUNK * SB], bf16, isOutput=False)
    xl_ext = nc.declare_dram_parameter("xl", [MPC, KX, NCHUNK * N], fp16, isOutput=False)
    xr_ext = nc.declare_dram_parameter("xr", [MPC, KX, NCHUNK * SB], fp16, isOutput=False)
    grep_ext = nc.declare_dram_parameter("grep4", [MPC, N, 2 * SB], bf16, isOutput=False)
    gtb_ext = nc.declare_dram_parameter("gtb", [MPC, N, N], bf16, isOutput=False)
    out_ext = nc.declare_dram_parameter("out", [MPC, N], f32, isOutput=True)

    from concourse.hw_specs import get_activation_tables

    _tables = get_activation_tables(nc.m.arch)
    _sig_id = next(
        i for i, (nm, fs) in enumerate(_tables.items())
        if F.Sigmoid in fs
    )

    with ExitStack() as ctx:
        tc = ctx.enter_context(tile.TileContext(nc))
        consts = ctx.enter_context(tc.tile_pool(name="consts", bufs=1))
        molp = ctx.enter_context(tc.tile_pool(name="mol", bufs=2))
        bigp = ctx.enter_context(tc.tile_pool(name="big", bufs=2))
        psum_th = ctx.enter_context(tc.tile_pool(name="psum_th", bufs=4, space="PSUM"))
        psum_x = ctx.enter_context(tc.tile_pool(name="psum_x", bufs=3, space="PSUM"))
        psum_v = ctx.enter_context(tc.tile_pool(name="psum_v", bufs=1, space="PSUM"))
        gsp = ctx.enter_context(tc.tile_pool(name="gs", bufs=6))
        dltp = ctx.enter_context(tc.tile_pool(name="dlt", bufs=6))
        p8gp = ctx.enter_context(tc.tile_pool(name="p8g", bufs=6))

        _last_act = [None]

        def _chain(ins):
            from concourse.tile_rust import add_dep_helper
            if _last_act[0] is not None:
                add_dep_helper(ins, _last_act[0], sync=False, reason="act-order")
            _last_act[0] = ins

        def load_sig_table():
            inst = mybir.InstLoadActFuncSet(
                name=nc.get_next_instruction_name(), ins=[], outs=[],
                act_func_set_id=_sig_id,
            )
            bi = nc.scalar.add_instruction(inst)
            _chain(bi.ins)

        ones_bf = consts.tile([N, 1], bf16, tag="ones_bf")
        nc.vector.memset(ones_bf[:], 1.0)
        load_sig_table()

        mol_state = {}

        def emit_prologue(m):
            GTb = molp.tile([N, N], bf16, tag="GTb")
            nc.sync.dma_start(out=GTb[:], in_=gtb_ext[m])
            GREP = molp.tile([N, 2 * SB], bf16, tag="GREP")
            nc.sync.dma_start(out=GREP[:], in_=grep_ext[m])
            V = psum_v.tile([N, N], f32, tag="V")
            mol_state[m] = dict(GTb=GTb, GREP=GREP, V=V)

        def emit_part(m, g0, g1):
            if g0 == 0:
                emit_prologue(m)
            st = mol_state[m]
            nb = g1 - g0
            THL = bigp.tile([K1, nb * N], bf16, tag="THL")
            THR = bigp.tile([K1, nb * SB], bf16, tag="THR")
            XL = bigp.tile([KX, nb * N], fp16, tag="XL")
            XR = bigp.tile([KX, nb * SB], fp16, tag="XR")
            NQ = max(1, nb // 4)  # ~4-chunk DMA granules
            qn, qs = nb * N // NQ, nb * SB // NQ
            nc.sync.dma_start(out=XL[:], in_=xl_ext[m, :, g0 * N:g1 * N])
            for qd in range(NQ):
                nc.sync.dma_start(
                    out=XR[:, qd * qs:(qd + 1) * qs],
                    in_=xr_ext[m, :, g0 * SB + qd * qs:g0 * SB + (qd + 1) * qs])
                nc.sync.dma_start(
                    out=THL[:, qd * qn:(qd + 1) * qn],
                    in_=thl_ext[m, :, g0 * N + qd * qn:g0 * N + (qd + 1) * qn])
                nc.sync.dma_start(
                    out=THR[:, qd * qs:(qd + 1) * qs],
                    in_=thr_ext[m, :, g0 * SB + qd * qs:g0 * SB + (qd + 1) * qs])

            def emit_vr(g, P8G, base=0):
                for q in range(CHUNK_J):
                    iq = g + NCHUNK * q
                    nc.tensor.matmul(
                        out=st["V"][:, iq:iq + 1],
                        lhsT=P8G[:, base + q * N:base + (q + 1) * N],
                        rhs=st["GTb"][:, iq:iq + 1], start=True, stop=True)

            for gb in range(g0, g1, 2):
                dlt2 = dltp.tile([N, 2 * SB], fp16, tag="dlt2")
                for gg in range(2):
                    g = gb + gg
                    lo_n, lo_s = (g - g0) * N, (g - g0) * SB
                    WP = psum_x.tile([N, SB], f32, tag="WP")
                    nc.tensor.matmul(
                        out=WP[:], lhsT=XL[:, lo_n:lo_n + N],
                        rhs=XR[:, lo_s:lo_s + SB], start=True, stop=True)
                    TH = psum_th.tile([N, SB], f32, tag="TH")
                    nc.tensor.matmul(
                        out=TH[:], lhsT=THL[:, lo_n:lo_n + N],
                        rhs=THR[:, lo_s:lo_s + SB], start=True, stop=True)
                    GS = gsp.tile([N, SB], f32, tag="GS")
                    bi = nc.scalar.activation(GS[:], WP[:], F.Sigmoid, scale=-1.0)
                    _chain(bi.ins)
                    nc.vector._custom_dve(
                        pa_op, out=dlt2[:, gg * SB:(gg + 1) * SB],
                        in0=TH[:], in1=GS[:],
                        s0=MAGIC, s1=NEUTRAL, imm2=0.25)
                P8G2 = p8gp.tile([N, 2 * SB], bf16, tag="P8G2")
                nc.vector._custom_dve(
                    pg_op, out=P8G2[:], in0=dlt2[:], in1=st["GREP"][:],
                    s0=PCP, s1=PCQ, imm2=PCR)
                emit_vr(gb, P8G2, base=0)
                emit_vr(gb + 1, P8G2, base=SB)
            if g1 == NCHUNK:
                W3 = molp.tile([N, N], bf16, tag="W3")
                nc.vector.tensor_mul(out=W3[:], in0=st["V"][:], in1=st["GTb"][:])
                outr = molp.tile([N, N], f32, tag="outr")
                import bass_rust
                nc.gpsimd.partition_all_reduce(
                    outr[:], W3[:], N, bass_rust.ReduceOp.add)
                nc.sync.dma_start(out=out_ext[m], in_=outr[:1, :])

        HB = NCHUNK // 2
        for m in range(MPC):
            if m == 0:
                emit_part(m, 0, 4)
                emit_part(m, 4, HB)
            else:
                emit_part(m, 0, HB)
            emit_part(m, HB, NCHUNK)

    return nc


def _get_graph():
    global _GRAPH
    if _GRAPH is None:
        _GRAPH = build_graph()
        _GRAPH.finalize()
    return _GRAPH


def make_in_maps(d_cutoff, d, atom_coordinates):
    in_maps = []
    for c in range(NCORES):
        per_mol = [
            _host_precompute(
                np.asarray(d[c * MPC + m], dtype=np.float32),
                np.asarray(d_cutoff[c * MPC + m], dtype=np.float32),
                np.asarray(atom_coordinates[c * MPC + m], dtype=np.float32),
            )
            for m in range(MPC)
        ]
        im = {
            k: np.ascontiguousarray(np.stack([pm[k] for pm in per_mol]))
            for k in per_mol[0]
        }
        in_maps.append(im)
    return in_maps


def kernel(d_cutoff, d, atom_coordinates):
    from concourse.bass_utils import run_bass_kernel_spmd

    nc = _get_graph()
    in_maps = make_in_maps(d_cutoff, d, atom_coordinates)
    res = run_bass_kernel_spmd(nc, in_maps, list(range(NCORES)))
    out = np.concatenate(
        [res.results[i]["out"] for i in range(NCORES)], axis=0
    ).astype(np.float32)
    return out


# revision 70
# speedup vs baseline: 1.0577x; 1.0181x over previous
"""Trainium2 Bass kernel v3 for nn_AngularSymmetry.

Layout: j in partitions, (q_i, k) in free dim; chunk g covers i in
{g, g+32, g+64, g+96}.  Denominators fold into host-packed matmuls:

  MM1 (K=63): t0[j,(q,k)] = theta/(4pi*d_ij*d_ik) + c0, c0=0.25-NEUTRAL
  WMM (K=15, fp16 split): w[j,(q,k)] = ln(y_ij) + ln(y_ik), y = sqe/d
  ACT Sigmoid(scale=-1): GS = sigmoid(-w) = 1/(1 + y_ij*y_ik)  exactly
       the baseline's GS = 1/XP, via the log-separable form (scalar
       engine; sigmoid table set only, loaded once)
  DVE pass1 (ANGSYM_PA2): dlt = |t - round(t)| - 0.25,
       t = t0*GS + NEUTRAL   (x huge -> GS~0 -> t=NEUTRAL, whose p8
       equals the uniform-phase mean: unbiased chaotic-band handling)
  DVE pass2 (ANGSYM_PG): p8g = (((s+p)s+q)s+r) * GREPc3,  s = dlt^2
       monic cubic minimax fit of 2*cos(2*pi*dlt)^1.6 / c3; the c3
       leading coef is folded into GREPc3 = G_jk * c3 on the host.
       This replaces the Sin+Ln+Exp scalar passes AND the G_jk mul.
  PE reduce (per chunk, 4x): V[:,i_q] = sum_j p8g[j,(q,:)]*GT[:,i_q]
       (folds G_ij; V is a per-molecule [128,128] f32 PSUM)
  epilogue: W3 = V * GT (bf16) -> out_i = ones-matmul over k; DMA.
"""

import numpy as np

B, N = 16, 128
NCORES = 8
MPC = B // NCORES
EPS = 1e-5
SQE = float(np.sqrt(EPS))
FLOOR = 1e-9
NROW_TH = 21                   # 5*4 + const
NROW_X = 5                     # ones + 1 per q
K1 = 3 * NROW_TH               # 63
KX = 3 * NROW_X                # 15
CHUNK_J = 4
NCHUNK = N // CHUNK_J          # 32
SB = CHUNK_J * N               # 512
MAGIC = 12582912.0             # 1.5*2^23
NEUTRAL = 0.119140625          # bf16-exact; p8(0.25-NEUTRAL) = chaotic mean
C0VAL = 0.130859375            # 0.25 - NEUTRAL, bf16-exact

# monic cubic fit of H(s)=2*cos(2*pi*sqrt(s))^1.6 on s in [0,1/16]:
# H(s) ~ PC3*(s^3 + PCP*s^2 + PCQ*s + PCR); minimax err ~2.8e-3
PC3 = -507.46850634104544
PCP = -1.0095257669475188
PCQ = 0.12224718009976722
PCR = -0.003935608640825085

_PA_OP = None
_PG_OP = None
_GRAPH = None


def _register_op(name, spec, rd1_en):
    from concourse import dve_ops
    from concourse.dve_ops import DveOp
    from concourse.dve_spec import lower
    from concourse.dve_uop import DveOpSpec

    for op in dve_ops.OPS:
        if op.name == name:
            return op
    opcode = max(dve_ops._SUB_OPCODE_FOR_NAME.values()) + 1
    assert opcode < 0x20
    dve_ops._SUB_OPCODE_FOR_NAME[name] = opcode
    shas = {}
    for ver in ("v3", "v4"):
        try:
            uops = lower(spec, ver=ver)
            shas[ver] = DveOpSpec(
                name=name, opcode=opcode, uops=uops, rd1_en=rd1_en
            ).sha(ver)
        except Exception:
            pass
    assert shas, f"{name} failed to lower for all DVE versions"
    op = DveOp(name, spec, subdim=False, uops_sha=shas)
    dve_ops.OPS.append(op)
    dve_ops.CUSTOM_DVE_SPECS[name] = spec
    return op


def _make_pa_op():
    """dlt = |t - round(t)| - imm2, t = in0*in1 + s1, MAGIC in s0."""
    global _PA_OP
    if _PA_OP is not None:
        return _PA_OP
    from concourse.dve_spec import AluOp, Bin, C0, C1, Spec, Src0, Src1

    t = Src0 * Src1 + C1
    t2 = t + C0
    kk = t2 - C0
    dd = Bin(AluOp.ABSOLUTE_DIFF, t, kk)
    from concourse.dve_spec import C2
    body = dd - C2

    def _ref(in0, in1, s0, s1, imm2):
        f32 = np.float32
        t = (in0.astype(f32) * in1.astype(f32) + f32(s1)).astype(f32)
        t2 = (t + f32(s0)).astype(f32)
        kk = (t2 - f32(s0)).astype(f32)
        return (np.abs((t - kk).astype(f32)) - f32(imm2)).astype(f32)

    _PA_OP = _register_op("ANGSYM_PA2", Spec(body=body, reference=_ref), True)
    return _PA_OP


def _make_pg_op():
    """p8g = (((s+s0)*s+s1)*s+imm2) * in1, s = in0^2."""
    global _PG_OP
    if _PG_OP is not None:
        return _PG_OP
    from concourse.dve_spec import C0, C1, C2, Spec, Src0, Src1, sq

    s = sq(Src0)
    h = ((s + C0) * s + C1) * s + C2
    body = h * Src1

    def _ref(in0, in1, s0, s1, imm2):
        f32 = np.float32
        ss = (in0.astype(f32) * in0.astype(f32)).astype(f32)
        h = (ss + f32(s0)).astype(f32)
        h = (h * ss + f32(s1)).astype(f32)
        h = (h * ss + f32(imm2)).astype(f32)
        return (h * in1.astype(f32)).astype(f32)

    _PG_OP = _register_op("ANGSYM_PG", Spec(body=body, reference=_ref), True)
    return _PG_OP


def _host_precompute(d, dc, coords):
    """Pack per-molecule device feeds. d, dc: [N,N] f32; coords: [N,3]."""
    import ml_dtypes

    f32 = np.float32
    bf = ml_dtypes.bfloat16
    C = coords.astype(np.float64)
    S = (C @ C.T).astype(f32)
    diag = np.diag(S).copy()
    Cf = coords.astype(f32)
    G = (dc.astype(np.float64)
         * np.exp(-d.astype(np.float64) ** 2)).astype(f32)
    dcl = np.maximum(d.astype(f32), f32(FLOOR))
    rinv4 = (1.0 / (4.0 * np.pi * dcl)).astype(f32)
    rink = (1.0 / dcl).astype(f32)
    lny = np.log(f32(SQE) / dcl).astype(f32)   # in [-5.8, 15]; w = lny+lny

    L = np.zeros((NROW_TH, NCHUNK, N), f32)
    R = np.zeros((NROW_TH, NCHUNK, SB), f32)
    Lx = np.zeros((NROW_X, NCHUNK, N), f32)
    Rx = np.zeros((NROW_X, NCHUNK, SB), f32)
    Lx[0] = 1.0
    for g in range(NCHUNK):
        for q in range(CHUNK_J):
            i = g + NCHUNK * q
            ks = slice(q * N, (q + 1) * N)
            r0 = 5 * q
            L[r0 + 0, g, :] = (diag[i] - S[i, :]) * rinv4[i, :]
            R[r0 + 0, g, ks] = rink[i, :]
            for c in range(3):
                L[r0 + 1 + c, g, :] = Cf[:, c] * rinv4[i, :]
                R[r0 + 1 + c, g, ks] = Cf[:, c] * rink[i, :]
            L[r0 + 4, g, :] = rinv4[i, :]
            R[r0 + 4, g, ks] = -S[i, :] * rink[i, :]
            Lx[1 + q, g, :] = lny[i, :]
            Rx[1 + q, g, ks] = 1.0
            Rx[0, g, ks] = lny[i, :]
    L[20, :, :] = 1.0
    R[20, :, :] = C0VAL

    def split2(Lm, Rm, nr, dt):
        Lh = Lm.astype(dt)
        Ll = (Lm - Lh.astype(f32)).astype(dt)
        Rh = Rm.astype(dt)
        Rl = (Rm - Rh.astype(f32)).astype(dt)
        lhs = np.concatenate([Lh, Lh, Ll], axis=0)
        rhs = np.concatenate([Rh, Rl, Rh], axis=0)
        return (np.ascontiguousarray(lhs.reshape(3 * nr, NCHUNK * Lm.shape[2])),
                np.ascontiguousarray(rhs.reshape(3 * nr, NCHUNK * Rm.shape[2])))

    thl, thr = split2(L, R, NROW_TH, bf)
    xl, xr = split2(Lx, Rx, NROW_X, np.float16)
    g4 = np.tile((G * f32(PC3)).astype(bf), (1, 4 * CHUNK_J))
    return {
        "thl": thl, "thr": thr, "xl": xl, "xr": xr,
        "grep4": np.ascontiguousarray(g4),  # c3-scaled G_jk, pre-tiled 8x
        "gtb": np.ascontiguousarray(G.T).astype(bf),
    }


def emulate(d_cutoff, d, atom_coordinates):
    """Pure-numpy emulation of the device pipeline (for validation)."""
    import ml_dtypes

    bf = ml_dtypes.bfloat16
    f32 = np.float32
    f16 = np.float16
    out = np.zeros((B, N), f32)
    for b in range(B):
        pm = _host_precompute(
            np.asarray(d[b], f32), np.asarray(d_cutoff[b], f32),
            np.asarray(atom_coordinates[b], f32))
        thl = pm["thl"].astype(f32).reshape(K1, NCHUNK, N)
        thr = pm["thr"].astype(f32).reshape(K1, NCHUNK, SB)
        xl = pm["xl"].astype(f32).reshape(KX, NCHUNK, N)
        xr = pm["xr"].astype(f32).reshape(KX, NCHUNK, SB)
        GT = pm["gtb"].astype(f32)
        Gc3rep = pm["grep4"].astype(f32)[:, :SB]
        V = np.zeros((N, N), f32)
        for g in range(NCHUNK):
            TH = np.einsum('mj,mc->jc', thl[:, g], thr[:, g]).astype(f32)
            WPm = np.einsum('mj,mc->jc', xl[:, g], xr[:, g]).astype(f32)
            GS = (f32(1.0) / (1.0 + np.exp(WPm))).astype(f32)  # sigmoid(-w)
            t = (TH * GS + f32(NEUTRAL)).astype(f32)
            kk = ((t + f32(MAGIC)).astype(f32) - f32(MAGIC)).astype(f32)
            dlt = (np.abs(t - kk) - f32(0.25)).astype(f16)
            ss = (dlt.astype(f32) ** 2).astype(f32)
            h = (ss + f32(PCP)).astype(f32)
            h = (h * ss + f32(PCQ)).astype(f32)
            h = (h * ss + f32(PCR)).astype(f32)
            p8g = (h * Gc3rep).astype(bf).astype(f32)
            for q in range(CHUNK_J):
                iq = g + NCHUNK * q
                V[:, iq] = p8g[:, q * N:(q + 1) * N].T @ GT[:, iq]
        W3 = (V * GT).astype(bf).astype(f32)
        out[b] = W3.sum(axis=0)
    return out


def build_graph(cfg=None):
    from contextlib import ExitStack

    import concourse.bass as bass
    import concourse.tile as tile
    from concourse import bacc, mybir
    from concourse.alu_op_type import AluOpType as ALU

    f32 = mybir.dt.float32
    bf16 = mybir.dt.bfloat16
    fp16 = mybir.dt.float16
    F = mybir.ActivationFunctionType

    pa_op = _make_pa_op()
    pg_op = _make_pg_op()

    nc = bacc.Bacc()
    thl_ext = nc.declare_dram_parameter("thl", [MPC, K1, NCHUNK * N], bf16, isOutput=False)
    thr_ext = nc.declare_dram_parameter("thr", [MPC, K1, NCHUNK * SB], bf16, isOutput=False)
    xl_ext = nc.declare_dram_parameter("xl", [MPC, KX, NCHUNK * N], fp16, isOutput=False)
    xr_ext = nc.declare_dram_parameter("xr", [MPC, KX, NCHUNK * SB], fp16, isOutput=False)
    grep_ext = nc.declare_dram_parameter("grep4", [MPC, N, 4 * SB], bf16, isOutput=False)
    gtb_ext = nc.declare_dram_parameter("gtb", [MPC, N, N], bf16, isOutput=False)
    out_ext = nc.declare_dram_parameter("out", [MPC, N], f32, isOutput=True)

    from concourse.hw_specs import get_activation_tables

    _tables = get_activation_tables(nc.m.arch)
    _sig_id = next(
        i for i, (nm, fs) in enumerate(_tables.items())
        if F.Sigmoid in fs
    )

    with ExitStack() as ctx:
        tc = ctx.enter_context(tile.TileContext(nc))
        consts = ctx.enter_context(tc.tile_pool(name="consts", bufs=1))
        molp = ctx.enter_context(tc.tile_pool(name="mol", bufs=2))
        bigp = ctx.enter_context(tc.tile_pool(name="big", bufs=2))
        psum_th = ctx.enter_context(tc.tile_pool(name="psum_th", bufs=4, space="PSUM"))
        psum_x = ctx.enter_context(tc.tile_pool(name="psum_x", bufs=3, space="PSUM"))
        psum_v = ctx.enter_context(tc.tile_pool(name="psum_v", bufs=1, space="PSUM"))
        gsp = ctx.enter_context(tc.tile_pool(name="gs", bufs=6))
        dltp = ctx.enter_context(tc.tile_pool(name="dlt", bufs=6))
        p8gp = ctx.enter_context(tc.tile_pool(name="p8g", bufs=6))

        _last_act = [None]

        def _chain(ins):
            from concourse.tile_rust import add_dep_helper
            if _last_act[0] is not None:
                add_dep_helper(ins, _last_act[0], sync=False, reason="act-order")
            _last_act[0] = ins

        def load_sig_table():
            inst = mybir.InstLoadActFuncSet(
                name=nc.get_next_instruction_name(), ins=[], outs=[],
                act_func_set_id=_sig_id,
            )
            bi = nc.scalar.add_instruction(inst)
            _chain(bi.ins)

        ones_bf = consts.tile([N, 1], bf16, tag="ones_bf")
        nc.vector.memset(ones_bf[:], 1.0)
        load_sig_table()

        mol_state = {}

        def emit_prologue(m):
            GTb = molp.tile([N, N], bf16, tag="GTb")
            nc.sync.dma_start(out=GTb[:], in_=gtb_ext[m])
            GREP = molp.tile([N, 4 * SB], bf16, tag="GREP")
            nc.sync.dma_start(out=GREP[:], in_=grep_ext[m])
            V = psum_v.tile([N, N], f32, tag="V")
            mol_state[m] = dict(GTb=GTb, GREP=GREP, V=V)

        def emit_part(m, g0, g1):
            if g0 == 0:
                emit_prologue(m)
            st = mol_state[m]
            nb = g1 - g0
            THL = bigp.tile([K1, nb * N], bf16, tag="THL")
            THR = bigp.tile([K1, nb * SB], bf16, tag="THR")
            XL = bigp.tile([KX, nb * N], fp16, tag="XL")
            XR = bigp.tile([KX, nb * SB], fp16, tag="XR")
            NQ = max(1, nb // 4)  # ~4-chunk DMA granules
            qn, qs = nb * N // NQ, nb * SB // NQ
            nc.sync.dma_start(out=XL[:], in_=xl_ext[m, :, g0 * N:g1 * N])
            for qd in range(NQ):
                nc.sync.dma_start(
                    out=XR[:, qd * qs:(qd + 1) * qs],
                    in_=xr_ext[m, :, g0 * SB + qd * qs:g0 * SB + (qd + 1) * qs])
                nc.sync.dma_start(
                    out=THL[:, qd * qn:(qd + 1) * qn],
                    in_=thl_ext[m, :, g0 * N + qd * qn:g0 * N + (qd + 1) * qn])
                nc.sync.dma_start(
                    out=THR[:, qd * qs:(qd + 1) * qs],
                    in_=thr_ext[m, :, g0 * SB + qd * qs:g0 * SB + (qd + 1) * qs])

            def emit_vr(g, P8G, base=0):
                for q in range(CHUNK_J):
                    iq = g + NCHUNK * q
                    nc.tensor.matmul(
                        out=st["V"][:, iq:iq + 1],
                        lhsT=P8G[:, base + q * N:base + (q + 1) * N],
                        rhs=st["GTb"][:, iq:iq + 1], start=True, stop=True)

            for gb in range(g0, g1, 4):
                dlt2 = dltp.tile([N, 4 * SB], fp16, tag="dlt2")
                for gg in range(4):
                    g = gb + gg
                    lo_n, lo_s = (g - g0) * N, (g - g0) * SB
                    WP = psum_x.tile([N, SB], f32, tag="WP")
                    nc.tensor.matmul(
                        out=WP[:], lhsT=XL[:, lo_n:lo_n + N],
                        rhs=XR[:, lo_s:lo_s + SB], start=True, stop=True)
                    TH = psum_th.tile([N, SB], f32, tag="TH")
                    nc.tensor.matmul(
                        out=TH[:], lhsT=THL[:, lo_n:lo_n + N],
                        rhs=THR[:, lo_s:lo_s + SB], start=True, stop=True)
                    GS = gsp.tile([N, SB], f32, tag="GS")
                    bi = nc.scalar.activation(GS[:], WP[:], F.Sigmoid, scale=-1.0)
                    _chain(bi.ins)
                    nc.vector._custom_dve(
                        pa_op, out=dlt2[:, gg * SB:(gg + 1) * SB],
                        in0=TH[:], in1=GS[:],
                        s0=MAGIC, s1=NEUTRAL, imm2=0.25)
                P8G2 = p8gp.tile([N, 4 * SB], bf16, tag="P8G2")
                nc.vector._custom_dve(
                    pg_op, out=P8G2[:], in0=dlt2[:], in1=st["GREP"][:],
                    s0=PCP, s1=PCQ, imm2=PCR)
                for gg in range(4):
                    emit_vr(gb + gg, P8G2, base=gg * SB)
            if g1 == NCHUNK:
                W3 = molp.tile([N, N], bf16, tag="W3")
                nc.vector.tensor_mul(out=W3[:], in0=st["V"][:], in1=st["GTb"][:])
                outr = molp.tile([N, N], f32, tag="outr")
                import bass_rust
                nc.gpsimd.partition_all_reduce(
                    outr[:], W3[:], N, bass_rust.ReduceOp.add)
                nc.sync.dma_start(out=out_ext[m], in_=outr[:1, :])

        HB = NCHUNK // 2
        for m in range(MPC):
            if m == 0:
                emit_part(m, 0, 4)
                emit_part(m, 4, HB)
            else:
                emit_part(m, 0, HB)
            emit_part(m, HB, NCHUNK)

    return nc


def _get_graph():
    global _GRAPH
    if _GRAPH is None:
        _GRAPH = build_graph()
        _GRAPH.finalize()
    return _GRAPH


def make_in_maps(d_cutoff, d, atom_coordinates):
    in_maps = []
    for c in range(NCORES):
        per_mol = [
            _host_precompute(
                np.asarray(d[c * MPC + m], dtype=np.float32),
                np.asarray(d_cutoff[c * MPC + m], dtype=np.float32),
                np.asarray(atom_coordinates[c * MPC + m], dtype=np.float32),
            )
            for m in range(MPC)
        ]
        im = {
            k: np.ascontiguousarray(np.stack([pm[k] for pm in per_mol]))
            for k in per_mol[0]
        }
        in_maps.append(im)
    return in_maps


def kernel(d_cutoff, d, atom_coordinates):
    from concourse.bass_utils import run_bass_kernel_spmd

    nc = _get_graph()
    in_maps = make_in_maps(d_cutoff, d, atom_coordinates)
    res = run_bass_kernel_spmd(nc, in_maps, list(range(NCORES)))
    out = np.concatenate(
        [res.results[i]["out"] for i in range(NCORES)], axis=0
    ).astype(np.float32)
    return out
